# revision 6
# baseline (speedup 1.0000x reference)
"""Dilated attention (LongNet-style) Bass kernel for 8 Trainium2 NeuronCores.

v2: fp32r matmuls (20-bit: 8-bit exp, 11+1-bit mantissa, full-rate on PE)
replace the fp16 hi/lo splitting of v1 where precision allows.

Problem: q,k,v of shape (B=2, S=8192, H=16, D=64) fp32.
4 head-groups x (segment length s, dilation r) with s/r == 1024 for every
group, so the whole computation is 120 identical 1024x1024x64 attention
sub-problems plus a per-(batch, head, channel) sum-normalization.

Sharding: core = b*4 + j owns heads {j, 4+j, 8+j, 12+j} of batch b, i.e.
one head from each group -> 8+4+2+1 = 15 sub-problems per core (perfectly
balanced), and every (batch, head) lives on exactly one core so the
normalization is core-local.

Numerics (from numpy simulation of the rounding schemes): the final
x / sum(x) normalization amplifies coherent per-(key,channel) errors.
K and V quantization errors are coherent across queries, so both use
hi+lo fp32r splits; E and Q rounding errors average out, single fp32r
is fine. Predicted L2 rel err ~8e-3 (vs 1.7e-2 for the fp16 v1 scheme).

Per sub-problem on-device (fp32 PSUM accumulation):
  S^T[k,q] = [khi;klo].T [q;q]           (one stacked K=128 fp32r matmul/tile)
  E        = exp(S^T) in fp32r           (softmax scale folded into q)
  O'[d,q]  = [Vhi|1].T E + [Vlo|0].T E   (fp32r; row 64 = denominator l)
  r        = 1/l via [128,8] DRAM detour (exact DVE recip, ~0.2us)
  x        = O'[0:64] * bcast(r)         (bcast via DRAM round-trip DMA)
  out      = x / (4 * sum_{segs,q} x)    per (head, channel)
"""

import os
import numpy as np

import concourse.bass as bass
import concourse.bacc as bacc
import concourse.mybir as mybir
import concourse.tile as tile
from concourse import bass_utils

# ---------------------------------------------------------------- constants
B, S, H, D = 2, 8192, 16, 64
SEGMENT_LENGTHS = [1024, 2048, 4096, 8192]
DILATION_RATES = [1, 2, 4, 8]
NUM_GROUPS = 4
GROUP_HEADS = H // NUM_GROUPS  # 4
SEGS_PER_GROUP = [S // s for s in SEGMENT_LENGTHS]  # [8, 4, 2, 1]
NPROB = sum(SEGS_PER_GROUP)  # 15 problems per core
SL = 1024          # per-problem sequence length (s // r, same for all groups)
NCHUNK = SL // 128  # 8 key chunks
N_CORES = 8
SCALE = 1.0 / np.sqrt(D)

FP32 = mybir.dt.float32
FP32R = mybir.dt.float32r
VW = D + 1  # 65: V plus the ones column


def r32r(x):
    """Round fp32 to fp32r (11 stored mantissa bits, RNE on the fp32 word)."""
    u = np.ascontiguousarray(np.asarray(x, np.float32)).view(np.uint32)
    low = u & np.uint32(0xFFF)
    half = np.uint32(0x800)
    u2 = u & np.uint32(0xFFFFF000)
    rnd = (low > half) | ((low == half) & ((u >> 12) & 1).astype(bool))
    u2 = u2 + (rnd.astype(np.uint32) << 12)
    return u2.view(np.float32)


def _problem_list(j):
    """15 (group, head, seg) tuples for local head-slot j, head-contiguous."""
    out = []
    for g in range(NUM_GROUPS):
        head = g * GROUP_HEADS + j
        for seg in range(SEGS_PER_GROUP[g]):
            out.append((g, head, seg))
    return out


def _positions(g, seg):
    s, r = SEGMENT_LENGTHS[g], DILATION_RATES[g]
    offset = g % r
    return seg * s + offset + r * np.arange(SL)


# ---------------------------------------------------------------- device IR
def _build_tile_program(ctx, tc, out_ap, qq_ap, kk_ap, vp_ap):
    nc = tc.nc
    EXP = mybir.ActivationFunctionType.Exp

    q_pool = ctx.enter_context(tc.tile_pool(name="qq", bufs=3))
    k_pool = ctx.enter_context(tc.tile_pool(name="kk", bufs=3))
    vp_pool = ctx.enter_context(tc.tile_pool(name="vp", bufs=3))
    exp_pool = ctx.enter_context(tc.tile_pool(name="exps", bufs=3))
    sout_pool = ctx.enter_context(tc.tile_pool(name="sout", bufs=3))
    snorm_pool = ctx.enter_context(tc.tile_pool(name="snorm", bufs=11))
    rrow_pool = ctx.enter_context(tc.tile_pool(name="rrow", bufs=3))
    sums_pool = ctx.enter_context(tc.tile_pool(name="sums", bufs=6))
    fin_pool = ctx.enter_context(tc.tile_pool(name="fin", bufs=3))
    rlb_pool = ctx.enter_context(tc.tile_pool(name="rlb", bufs=2))
    lt_pool = ctx.enter_context(tc.tile_pool(name="lt", bufs=4))
    rdram_pool = ctx.enter_context(
        tc.tile_pool(name="rdram", bufs=3, space="DRAM"))
    spsum = ctx.enter_context(tc.tile_pool(name="spsum", bufs=2, space="PSUM"))
    pvpsum = ctx.enter_context(tc.tile_pool(name="pvpsum", bufs=2, space="PSUM"))

    # per-problem state; problems are head-contiguous
    probs = []
    for g in range(NUM_GROUPS):
        for seg in range(SEGS_PER_GROUP[g]):
            probs.append({
                "first": seg == 0,
                "last": seg == SEGS_PER_GROUP[g] - 1,
            })
    for p, st in enumerate(probs):
        st["p"] = p
    head_lists = []
    i = 0
    for nseg in SEGS_PER_GROUP:
        head_lists.append(probs[i:i + nseg])
        i += nseg
    for hl in head_lists:
        for st in hl:
            st["head_list"] = hl

    def emit_bcast(st):
        # 1/l + broadcast to 64 partitions, via DRAM round-trips:
        # l row was DMA'd to DRAM (ld) at problem end; reload as [128, 8],
        # exact-reciprocal there (cheap across 128 lanes), store back to
        # DRAM, then stride-0 partition-broadcast to [64, SL].
        ld = st["l_dram"]
        lt = lt_pool.tile([128, SL // 128], FP32)
        src = bass.AP(tensor=ld.tensor, offset=ld.offset,
                      ap=[[SL // 128, 128], [1, SL // 128]])
        nc.gpsimd.dma_start(out=lt, in_=src)
        rt = lt_pool.tile([128, SL // 128], FP32, tag="rt")
        nc.vector.reciprocal(out=rt, in_=lt)
        r_d = rdram_pool.tile([1, SL], FP32)
        dst = bass.AP(tensor=r_d.tensor, offset=r_d.offset,
                      ap=[[SL // 128, 128], [1, SL // 128]])
        nc.gpsimd.dma_start(out=dst, in_=rt)
        rl_b = rlb_pool.tile([D, SL], FP32)
        st["rl_b"] = rl_b
        src2 = bass.AP(tensor=r_d.tensor, offset=r_d.offset,
                       ap=[[0, D]] + [list(d) for d in r_d.ap[1:]])
        nc.gpsimd.dma_start(out=rl_b, in_=src2)

    def emit_norm(st):
        # s_norm = s_out[0:64] * bcast(1/l); seg_sum = sum_q s_norm + prev
        prev_accum = None if st["first"] else probs[st["p"] - 1]["seg_sum"]
        s_norm = snorm_pool.tile([D, SL], FP32)
        seg_local = sums_pool.tile([D, 1], FP32, tag="seg_local")
        nc.vector.affine_mul_reduce(
            out=s_norm, accum_out=seg_local,
            in0=st["s_out"][0:D, :], in1=st["rl_b"], scale=1.0, bias=0.0)
        if prev_accum is None:
            seg_sum = seg_local
        else:
            seg_sum = sums_pool.tile([D, 1], FP32, tag="seg_sum")
            nc.vector.tensor_add(seg_sum, seg_local, prev_accum)
        st["s_norm"] = s_norm
        st["seg_sum"] = seg_sum
        if st["last"]:
            emit_head_finals(st)

    pending_fins = []  # (st, rh) pairs not yet multiplied/DMA'd out

    def emit_head_finals(last_st):
        # rh = 1 / (4 * head_sum); the per-segment final muls are QUEUED and
        # spread across later problem boundaries (after each PSUM drain) so
        # this burst never delays the drain the PE is waiting on.
        hs4 = sums_pool.tile([D, 1], FP32)
        nc.vector.tensor_scalar_mul(hs4, last_st["seg_sum"], float(NUM_GROUPS))
        rh = sums_pool.tile([D, 1], FP32, tag="rh", bufs=2)
        nc.vector.reciprocal(out=rh, in_=hs4)
        for st in last_st["head_list"]:
            pending_fins.append((st, rh))

    def flush_fins(n):
        for _ in range(min(n, len(pending_fins))):
            st, rh = pending_fins.pop(0)
            fin = fin_pool.tile([D, SL], FP32)
            nc.vector.tensor_scalar_mul(fin, st["s_norm"], rh)
            nc.scalar.dma_start(out=out_ap[st["p"]], in_=fin)

    prev = None  # previous problem (epilogue pipelined one problem behind)

    reps = int(os.environ.get("DILATED_REPS", "1"))
    for p in [i % NPROB for i in range(reps * NPROB)]:
        st = probs[p]

        q_t = q_pool.tile([128, SL], FP32R)
        nc.sync.dma_start(out=q_t[0:D, :], in_=qq_ap[p])
        nc.sync.dma_start(out=q_t[D:2 * D, :], in_=qq_ap[p])
        k_t = k_pool.tile([128, SL], FP32R)
        nc.sync.dma_start(out=k_t, in_=kk_ap[p])
        vp_t = vp_pool.tile([128, NCHUNK * 2 * VW], FP32R)
        nc.scalar.dma_start(out=vp_t, in_=vp_ap[p])

        pv_ps = None
        e_tiles = [None] * NCHUNK
        for c in range(NCHUNK + 1):
            if c < NCHUNK:
                # S^T chunk c = [khi;klo].T @ [q;q]  (one fp32r matmul/half)
                s_ps = spsum.tile([128, SL], FP32)
                for h in range(2):
                    hs = slice(h * 512, (h + 1) * 512)
                    nc.tensor.matmul(
                        out=s_ps[:, hs],
                        lhsT=k_t[:, c * 128: (c + 1) * 128],
                        rhs=q_t[:, hs],
                        start=True, stop=True,
                    )

                if c == 1 and prev is not None:
                    emit_bcast(prev)
                if c == 2 and prev is not None:
                    emit_norm(prev)

                e_t = exp_pool.tile([128, SL], FP32R)
                nc.scalar.activation(out=e_t, in_=s_ps, func=EXP)
                e_tiles[c] = e_t

            if c >= 1:
                # PV for chunk c-1 (one chunk behind so PE never waits on ACT)
                cc = c - 1
                e_t = e_tiles[cc]
                if pv_ps is None:
                    pv_ps = pvpsum.tile([128, SL], FP32, tag="pv")
                for h in range(2):
                    hs = slice(h * 512, (h + 1) * 512)
                    nc.tensor.matmul(      # [Vhi | 1].T @ E
                        out=pv_ps[0:VW, hs],
                        lhsT=vp_t[:, cc * 2 * VW: cc * 2 * VW + VW],
                        rhs=e_t[:, hs],
                        start=(cc == 0), stop=False,
                    )
                    nc.tensor.matmul(      # [Vlo | 0].T @ E
                        out=pv_ps[0:VW, hs],
                        lhsT=vp_t[:, cc * 2 * VW + VW: (cc + 1) * 2 * VW],
                        rhs=e_t[:, hs],
                        start=False, stop=(cc == NCHUNK - 1),
                    )

        # evacuate PV psum fast (frees the slot), compute 1/l row
        s_out = sout_pool.tile([VW, SL], FP32)
        nc.vector.tensor_copy(out=s_out, in_=pv_ps[0:VW, :])
        st["s_out"] = s_out
        l_dram = rdram_pool.tile([1, SL], FP32, tag="l_dram")
        nc.gpsimd.dma_start(out=l_dram, in_=s_out[D:D + 1, :])
        st["l_dram"] = l_dram

        flush_fins(2)

        prev = st

    # drain the last problem's epilogue
    emit_bcast(prev)
    emit_norm(prev)
    flush_fins(len(pending_fins))


# Cache: the Bass program is identical for every call (and every core).
_CACHED = {}


def _get_program():
    key = os.environ.get("DILATED_REPS", "1")
    if key in _CACHED:
        return _CACHED[key]
    nc = bacc.Bacc("TRN2", target_bir_lowering=False, debug=False)
    qq = nc.dram_tensor("qq", [NPROB, D, SL], FP32R,
                        kind="ExternalInput").ap()
    kk = nc.dram_tensor("kk", [NPROB, 128, SL], FP32R,
                        kind="ExternalInput").ap()
    vp = nc.dram_tensor("vp", [NPROB, 128, NCHUNK * 2 * VW], FP32R,
                        kind="ExternalInput").ap()
    out = nc.dram_tensor("out", [NPROB, D, SL], FP32, kind="ExternalOutput").ap()
    from contextlib import ExitStack
    with tile.TileContext(nc) as tc, ExitStack() as ctx:
        _build_tile_program(ctx, tc, out, qq, kk, vp)
    nc.compile()
    _CACHED[key] = nc
    return nc


# ---------------------------------------------------------------- host glue
def _prep_core(q, k, v, b, j):
    """Build the qq/kk/vp device inputs for core (b, j). q is pre-scaled."""
    qq = np.empty((NPROB, D, SL), dtype=np.float32)
    kk = np.empty((NPROB, 128, SL), dtype=np.float32)
    vp = np.empty((NPROB, 128, NCHUNK * 2 * VW), dtype=np.float32)
    ones = np.ones((SL, 1), np.float32)
    zeros = np.zeros((SL, 1), np.float32)
    for p, (g, head, seg) in enumerate(_problem_list(j)):
        pos = _positions(g, seg)
        qq[p] = r32r(q[b, pos, head, :].T)  # [64, 1024], scaled
        kT = k[b, pos, head, :].T
        khi = r32r(kT)
        kk[p, 0:D, :] = khi
        kk[p, D:2 * D, :] = r32r(kT - khi)
        vs = v[b, pos, head, :]  # [1024, 64] fp32
        vhi = r32r(vs)
        vlo = r32r(vs - vhi)
        vfull = np.concatenate([vhi, ones, vlo, zeros], axis=1)  # [1024, 130]
        vp[p] = (vfull.reshape(NCHUNK, 128, 2 * VW)
                 .transpose(1, 0, 2).reshape(128, NCHUNK * 2 * VW))
    return {"qq": qq, "kk": kk, "vp": vp}


def kernel(query, key, value, _run_kw=None):
    q = np.asarray(query, dtype=np.float32)
    k = np.asarray(key, dtype=np.float32)
    v = np.asarray(value, dtype=np.float32)
    qs = q * SCALE  # fold softmax scale into q

    nc = _get_program()
    in_maps = []
    core_meta = []
    for core in range(N_CORES):
        b, j = divmod(core, NUM_GROUPS)
        in_maps.append(_prep_core(qs, k, v, b, j))
        core_meta.append((b, j))

    kw = dict(_run_kw or {})
    kw.pop("result", None)
    res = bass_utils.run_bass_kernel_spmd(
        nc, in_maps, core_ids=list(range(N_CORES)), **kw)

    out = np.zeros((B, S, H, D), dtype=np.float32)
    for core in range(N_CORES):
        b, j = core_meta[core]
        dev_out = res.results[core]["out"]  # [15, 64, 1024] fp32
        for p, (g, head, seg) in enumerate(_problem_list(j)):
            pos = _positions(g, seg)
            out[b, pos, head, :] = dev_out[p].T
    if _run_kw is not None:
        _run_kw["result"] = res
    return out
